# revision 3
# baseline (speedup 1.0000x reference)
"""Trainium2 Bass kernel for nn_CAM_62852551409742.

Math (reference):
  f = feats[:, :, 0, :]                               [R,B,T], R=4, B=512, T=150
  feat_n = feats.reshape(B, R*T)                      [B,K], K=600
  att[r,b,t,k] = tanh(a[r]*f[r,b,t] * feat_n[b,k])
  Hm = relu(att @ Wc[r].T + f*W[r])                   [R,B,T,32]
  attf = Hm @ Wh[r] + f                               [R,B,T]
  ff[b, r*T+t] = attf[r,b,t]
  out = (ff @ W1.T + b1) @ W2.T + b2                  [B,1,7]

Strategy: data-parallel over B across 8 cores (64 batches each). On device,
per 8-batch group: DVE builds z[k,(b,r,t)] = af broadcast * fn column
(tensor_scalar, 4x bf16), ACT applies tanh in place with huge free dims,
PE contracts k against Wc^T tiles into PSUM [(r,c) x (b,t)] chunks
(f*W folded in as an extra contraction row on the last k-tile), DVE relu ->
Hm_all bf16. Final: the linear tail is algebraically collapsed on the host
(Wx = W2@W1, U[(r,c),t,i] = Wh[r,c]*Wx[i,r*T+t]) so 150 small matmuls
(lhsT = Hm slice, rhs = U_t) plus 5 fp32 matmuls (f^T x Wx^T) accumulate the
final [64,7] directly in PSUM.
"""

from contextlib import ExitStack

import numpy as np
import ml_dtypes

import concourse.bacc as bacc
import concourse.bass as bass
import concourse.tile as tile
from concourse import mybir
from concourse import bass_utils

R, B, T, H = 4, 512, 150, 32
K = R * T                      # 600
NCORES = 8
BL = B // NCORES               # 64 batches per core
G, GB = 8, 8                   # 8 groups of 8 batches
KTS = [(0, 128), (128, 128), (256, 128), (384, 128), (512, 88)]
F32 = mybir.dt.float32
BF16 = mybir.dt.bfloat16
BF = ml_dtypes.bfloat16

_CACHE = {}


def build_nc():
    nc = bacc.Bacc("TRN2", target_bir_lowering=False)
    af_d = nc.dram_tensor("af", [BL, K], BF16, kind="ExternalInput")
    f_d = nc.dram_tensor("fr", [G, GB, K], BF16, kind="ExternalInput")
    fn_d = nc.dram_tensor("fn", [128, 5, BL], F32, kind="ExternalInput")
    wc_d = nc.dram_tensor("wc", [128, R, 5, H], BF16, kind="ExternalInput")
    u_d = nc.dram_tensor("u", [128, T, 7], BF16, kind="ExternalInput")
    ft_d = nc.dram_tensor("ft", [128, 5, BL], F32, kind="ExternalInput")
    wx_d = nc.dram_tensor("wx", [128, 5, 7], F32, kind="ExternalInput")
    bx_d = nc.dram_tensor("bx", [7], F32, kind="ExternalInput")
    out_d = nc.dram_tensor("out", [BL, 7], F32, kind="ExternalOutput")

    with tile.TileContext(nc) as tc, ExitStack() as ctx:
        consts = ctx.enter_context(tc.tile_pool(name="consts", bufs=1))
        attp = ctx.enter_context(tc.tile_pool(name="att", bufs=2))
        afp = ctx.enter_context(tc.tile_pool(name="afp", bufs=2))
        hmp = ctx.enter_context(tc.tile_pool(name="hm", bufs=1))
        outp = ctx.enter_context(tc.tile_pool(name="outp", bufs=1))
        psum = ctx.enter_context(tc.tile_pool(name="ps", bufs=6, space="PSUM"))
        psum_o = ctx.enter_context(tc.tile_pool(name="pso", bufs=1, space="PSUM"))

        wc_sb = consts.tile([128, R, 5, H], BF16)
        nc.sync.dma_start(out=wc_sb[:], in_=wc_d[:])
        fn_sb = consts.tile([128, 5, BL], F32)
        nc.sync.dma_start(out=fn_sb[:], in_=fn_d[:])
        u_sb = consts.tile([128, T, 7], BF16)
        nc.sync.dma_start(out=u_sb[:], in_=u_d[:])
        ft_sb = consts.tile([128, 5, BL], F32)
        nc.sync.dma_start(out=ft_sb[:], in_=ft_d[:])
        wx_sb = consts.tile([128, 5, 7], F32)
        nc.sync.dma_start(out=wx_sb[:], in_=wx_d[:])
        bx_sb = consts.tile([BL, 7], F32)
        nc.sync.dma_start(
            out=bx_sb[:],
            in_=bass.AP(tensor=bx_d, offset=0, ap=[[0, BL], [1, 7]]),
        )
        hm_all = hmp.tile([128, BL, T], BF16)

        chunks = [(0, 3), (3, 3), (6, 2)]
        for g in range(G):
            af_g = afp.tile([128, GB, K], BF16)
            for b in range(GB):
                nc.sync.dma_start(
                    out=af_g[:, b, :],
                    in_=bass.AP(
                        tensor=af_d, offset=(g * GB + b) * K, ap=[[0, 128], [1, K]]
                    ),
                )
            atts = []
            for kt, (k0, kp) in enumerate(KTS):
                at = attp.tile([128, GB, K], BF16, tag=f"att{kt}")
                atts.append(at)
                if kt == 4:
                    nc.sync.dma_start(out=at[88:89, :, :], in_=f_d[g : g + 1, :, :])
                for b in range(GB):
                    nc.vector.tensor_scalar_mul(
                        out=at[0:kp, b, :],
                        in0=af_g[0:kp, b, :],
                        scalar1=fn_sb[0:kp, kt, g * GB + b : g * GB + b + 1],
                    )
                nc.scalar.activation(
                    out=at[0:kp, :, :],
                    in_=at[0:kp, :, :],
                    func=mybir.ActivationFunctionType.Tanh,
                )
            ptiles = []
            for ci, (_, nb) in enumerate(chunks):
                pt = psum.tile([128, nb, T], F32, tag="hmps")
                ptiles.append(pt)
            for r in range(R):
                for kt, (k0, kp) in enumerate(KTS):
                    pp = kp + 1 if kt == 4 else kp
                    lhsT = wc_sb[0:pp, r, kt, :]
                    for ci, (s, nb) in enumerate(chunks):
                        nc.tensor.matmul(
                            out=ptiles[ci][r * H : (r + 1) * H, :, :],
                            lhsT=lhsT,
                            rhs=atts[kt][0:pp, s : s + nb, r * T : (r + 1) * T],
                            start=(kt == 0),
                            stop=(kt == 4),
                            tile_position=(0, r * H),
                        )
            for ci, (s, nb) in enumerate(chunks):
                nc.vector.tensor_scalar_max(
                    out=hm_all[:, g * GB + s : g * GB + s + nb, :],
                    in0=ptiles[ci][:],
                    scalar1=0.0,
                )

        op = psum_o.tile([BL, 7], F32)
        for t in range(T):
            nc.tensor.matmul(
                out=op[:],
                lhsT=hm_all[:, :, t],
                rhs=u_sb[:, t, :],
                start=(t == 0),
                stop=False,
            )
        for kt, (k0, kp) in enumerate(KTS):
            nc.tensor.matmul(
                out=op[:],
                lhsT=ft_sb[0:kp, kt, :],
                rhs=wx_sb[0:kp, kt, :],
                start=False,
                stop=(kt == 4),
            )
        ob = outp.tile([BL, 7], F32)
        nc.vector.tensor_add(out=ob[:], in0=op[:], in1=bx_sb[:])
        nc.sync.dma_start(out=out_d[:], in_=ob[:])

    nc.finalize()
    return nc


def _host_prep(feats, a, W, Wc, Wh, W1, b1, W2, b2):
    """Per-core input maps. feats: [R,B,1,T] fp32."""
    f = feats[:, :, 0, :]                              # [R,B,T]
    af_full = a[:, None, None] * f                     # [R,B,T]
    feat_n = feats.reshape(B, K)                       # [B,K]
    Wx = W2 @ W1                                       # [7,K]
    bx = W2 @ b1 + b2                                  # [7]

    # U[(r,c), t, i] = Wh[r,c] * Wx[i, r*T+t]
    U = np.zeros((128, T, 7), np.float32)
    for r in range(R):
        blk = Wx[:, r * T : (r + 1) * T].T             # [T,7]
        U[r * H : (r + 1) * H] = Wh[r][:, None, None] * blk[None]

    # wc_h[p, r, kt, c]: Wc[r].T rows per k-tile; kt4 row 88 = W[r]
    wc_h = np.zeros((128, R, 5, H), np.float32)
    for r in range(R):
        for kt, (k0, kp) in enumerate(KTS):
            wc_h[:kp, r, kt, :] = Wc[r, :, k0 : k0 + kp].T
        wc_h[88, r, 4, :] = W[r]

    wx_h = np.zeros((128, 5, 7), np.float32)
    for kt, (k0, kp) in enumerate(KTS):
        wx_h[:kp, kt, :] = Wx[:, k0 : k0 + kp].T

    fT_full = np.concatenate([f[r].T for r in range(R)], axis=0)  # [K, B]

    in_maps = []
    for m in range(NCORES):
        b0 = m * BL
        af_h = np.ascontiguousarray(
            af_full[:, b0 : b0 + BL, :].transpose(1, 0, 2).reshape(BL, K)
        ).astype(BF)
        f_h = np.ascontiguousarray(
            f[:, b0 : b0 + BL, :].transpose(1, 0, 2).reshape(G, GB, K)
        ).astype(BF)
        fn_h = np.zeros((128, 5, BL), np.float32)
        for kt, (k0, kp) in enumerate(KTS):
            fn_h[:kp, kt, :] = feat_n[b0 : b0 + BL, k0 : k0 + kp].T
        ft_h = np.zeros((128, 5, BL), np.float32)
        for kt, (k0, kp) in enumerate(KTS):
            ft_h[:kp, kt, :] = fT_full[k0 : k0 + kp, b0 : b0 + BL]
        in_maps.append(
            {
                "af": af_h,
                "fr": f_h,
                "fn": fn_h,
                "wc": wc_h.astype(BF),
                "u": U.astype(BF),
                "ft": ft_h,
                "wx": wx_h,
                "bx": bx.astype(np.float32),
            }
        )
    return in_maps


def kernel(feats_list, a, W, Wc, Wh, W1, b1, W2, b2):
    feats = np.asarray(feats_list, np.float32)
    in_maps = _host_prep(
        feats,
        np.asarray(a, np.float32),
        np.asarray(W, np.float32),
        np.asarray(Wc, np.float32),
        np.asarray(Wh, np.float32),
        np.asarray(W1, np.float32),
        np.asarray(b1, np.float32),
        np.asarray(W2, np.float32),
        np.asarray(b2, np.float32),
    )
    if "nc" not in _CACHE:
        _CACHE["nc"] = build_nc()
    res = bass_utils.run_bass_kernel_spmd(
        _CACHE["nc"], in_maps, core_ids=list(range(NCORES))
    )
    _CACHE["last_result"] = res
    out = np.concatenate([r["out"] for r in res.results], axis=0)  # [B,7]
    return out[:, None, :].astype(np.float32)                      # [B,1,7]
